# revision 13
# baseline (speedup 1.0000x reference)
"""Trainium2 Bass kernel for nn_Attention_21809843929849 (sparse_attention).

The reference scatters the attention output into `out` and then immediately
overwrites the exact same rows with `x[i, L-1-topk_index[i]]` (the faithful
`~idx` bug from the original module). The attention math is therefore dead
code and the true computation is pure memory movement:

    out[i, j, :] = x[i, L-1-j, :]   if j in topk_index[i]
                 = 0                otherwise

Sharding: 8 cores = 4 batches x 2 halves of the sequence. Core c owns batch
c//2 and output rows [2048*(c%2), 2048*(c%2+1)). Because the source row of
output row j is L-1-j, each core only ever reads from the *opposite* half of
its batch's x, so each core gets an 8MB x-half as input. In region-local
coordinates r, the source row inside that half is simply 2047-r.

Both run_bass_kernel_spmd execution paths hand the NEFF pre-zeroed output
buffers (native run_neff pre-zeros out_maps; the axon/PJRT path donates
zero-initialized arrays as outputs — kernels that don't write every element
rely on this). So the kernel never writes the ~75% zero rows at all: the
host compacts the selected rows and the device only moves those.

Device kernel per core (raw Bass, same SPMD program on all 8 cores):
the <=768 selected rows are processed as 6 blocks of 128 (one row per SBUF
partition):
  - indirect-DMA-gather block b's source rows into SBUF tile b (gpsimd
    SWDGE, one row-offset per partition; out-of-bounds sentinel entries for
    padding are skipped),
  - indirect-DMA-scatter tile b to the selected output rows (dst offset per
    partition, sentinel entries skipped).

Raw Bass with explicit semaphores is used instead of the Tile framework:
this toolchain's walrus codegen only supports a single sync-wait command
per instruction, which the Tile auto-sync (multi-wait drains) violates.
The HW indirect DMA consumes ONE index per partition and moves the whole
per-partition free size contiguously from that offset, hence [P, 1] offset
slices per call.

Per-core HBM traffic: ~2MB gathered reads + ~2MB scattered writes.
"""

import numpy as np

B, L, D = 4, 4096, 1024
H = L // 2          # rows per core region
P = 128             # SBUF partitions
NB = 5              # compacted blocks of 128 rows; capacity 640 vs per-core
                    # counts ~512±14 (observed max 527 for the fixed seed)
SENTINEL = 10**6    # > bounds_check -> indirect entry skipped
N_CORES = 8

_compiled = None


def _build():
    import concourse.bass as bass
    from concourse import mybir

    nc = bass.Bass("TRN2", target_bir_lowering=False)
    x_in = nc.dram_tensor("x_in", [H, D], mybir.dt.float32, kind="ExternalInput")
    # columns 0..NB-1: source row offsets; columns NB..2NB-1: dst row offsets
    offs = nc.dram_tensor("offs", [P, 2 * NB], mybir.dt.int32, kind="ExternalInput")
    out = nc.dram_tensor("out", [H, D], mybir.dt.float32, kind="ExternalOutput")

    offs_sb = nc.alloc_sbuf_tensor("offs_sb", [P, 2 * NB], mybir.dt.int32)
    tiles = [
        nc.alloc_sbuf_tensor(f"tile{b}", [P, D], mybir.dt.float32)
        for b in range(NB)
    ]

    sem_o = nc.alloc_semaphore("sem_o")                       # offsets loaded
    sem_g = [nc.alloc_semaphore(f"sem_g{b}") for b in range(NB)]  # gathers landed
    sem_s = nc.alloc_semaphore("sem_s")                       # scatters landed

    with nc.Block() as blk:

        @blk.sync
        def _(sync):
            sync.dma_start(out=offs_sb[:], in_=offs[:]).then_inc(sem_o, 16)
            sync.wait_ge(sem_s, 16 * NB)

        @blk.gpsimd
        def _(pool):
            bc = pool.snap(H - 1)
            pool.wait_ge(sem_o, 16)
            for b in range(NB):
                pool.indirect_dma_start(
                    out=tiles[b][:],
                    out_offset=None,
                    in_=x_in[:],
                    in_offset=bass.IndirectOffsetOnAxis(
                        ap=offs_sb[:, b:b + 1], axis=0
                    ),
                    bounds_check=bc,
                    oob_is_err=False,
                ).then_inc(sem_g[b], 16)
            for b in range(NB):
                pool.wait_ge(sem_g[b], 16)
                pool.indirect_dma_start(
                    out=out[:],
                    out_offset=bass.IndirectOffsetOnAxis(
                        ap=offs_sb[:, NB + b:NB + b + 1], axis=0
                    ),
                    in_=tiles[b][:],
                    in_offset=None,
                    bounds_check=bc,
                    oob_is_err=False,
                ).then_inc(sem_s, 16)

    nc.finalize()
    return nc


LAST_RESULT = None  # BassKernelResults of the most recent run (for profiling)


def _make_offs(sel_half):
    """Compact the selected rows of one core region into the [P, 2*NB] int32
    offset table: entry e=b*128+p -> src column b = local source row 2047-r,
    dst column NB+b = output row r; padding entries get the OOB sentinel."""
    rows = np.flatnonzero(sel_half)
    assert len(rows) <= NB * P, (
        f"{len(rows)} selected rows exceed kernel capacity {NB * P}"
    )
    src = np.full(NB * P, SENTINEL, np.int32)
    dst = np.full(NB * P, SENTINEL, np.int32)
    src[: len(rows)] = (H - 1) - rows
    dst[: len(rows)] = rows
    return np.ascontiguousarray(
        np.concatenate(
            [src.reshape(NB, P).T, dst.reshape(NB, P).T], axis=1
        ).astype(np.int32)
    )


def kernel(x, Wq, Wk, Wv, select_x_mask, topk_index, _trace=False):
    from concourse.bass_utils import run_bass_kernel_spmd

    global _compiled, LAST_RESULT
    if _compiled is None:
        _compiled = _build()

    x = np.asarray(x, dtype=np.float32)
    topk = np.asarray(topk_index).astype(np.int64)

    row_mask = np.zeros((B, L), dtype=bool)
    row_mask[np.arange(B)[:, None], topk] = True

    in_maps = []
    for c in range(N_CORES):
        i, h = divmod(c, 2)
        offs_tiled = _make_offs(row_mask[i, h * H:(h + 1) * H])
        x_half = np.ascontiguousarray(x[i, (1 - h) * H:(2 - h) * H, :])
        in_maps.append({"x_in": x_half, "offs": offs_tiled})

    res = run_bass_kernel_spmd(
        _compiled, in_maps, core_ids=list(range(N_CORES)), trace=_trace
    )
    LAST_RESULT = res

    out_full = np.empty((B, L, D), dtype=np.float32)
    for c in range(N_CORES):
        i, h = divmod(c, 2)
        out_full[i, h * H:(h + 1) * H, :] = res.results[c]["out"]
    return out_full
